# revision 1
# baseline (speedup 1.0000x reference)
"""Trainium2 Bass kernel for GQA attention (dense_transformer).

Full module: x[1,2048,4096] -> causal GQA attention (32 q heads, 8 kv heads,
head_dim 128, RoPE) -> out[1,2048,4096].

Sharding: tensor-parallel by heads across 8 NeuronCores. Core c owns q heads
4c..4c+3 and kv head c; wq/wk/wv column-sharded, wo row-sharded; x replicated.
The trailing all-reduce over wo partial sums is done host-side (outputs are
gathered to host anyway).

On-chip layout notes:
  - All DRAM-side operands are pre-transposed on host so every matmul operand
    has its contraction dim on SBUF partitions with contiguous DMA patterns.
  - RoPE pairs are de-interleaved host-side (even rows then odd rows of each
    head of wq/wk), which turns the rotation into 64-partition-shifted
    multiply/adds on chip. Dot products are invariant to the permutation.
  - Scores are computed transposed (sk on partitions, sq on free) so the P@V
    matmul needs no on-chip transpose of the probabilities. The softmax
    denominator is accumulated with DVE adds and reduced across partitions
    with a GpSimd partition_all_reduce. Softmax max-subtraction is skipped:
    scores are O(±10) here, exp cannot overflow in fp32, and the result is
    identical up to rounding.
  - Matmuls run as float32r (full PE rate at free-dim >= 256).
"""

import math
from contextlib import ExitStack

import numpy as np

import concourse.bass as bass
import concourse.mybir as mybir
import concourse.tile as tile
from concourse import bacc, bass_isa, bass_utils

F32 = mybir.dt.float32
F32R = mybir.dt.float32r

# Full-scale config (hardcoded; kernel.py must be self-contained).
DIM = 4096
SEQ = 2048
N_HEADS = 32
N_KV_HEADS = 8
HEAD_DIM = 128
N_CORES = 8
HQ = N_HEADS // N_CORES            # q heads per core = 4
CH = 512                           # sq chunk (free dim of most matmuls)
SCALE = 1.0 / math.sqrt(HEAD_DIM)


def build_module(S=SEQ, D=DIM, hq=HQ, ch=CH, use_par_reduce=False):
    """Build the SPMD Bass/Tile module for one core's shard."""
    HD = HEAD_DIM
    H2 = HD // 2
    M = hq * HD                     # local q output dim
    R = ch // 128                   # sk-tiles per sq chunk
    nJ = S // ch                    # sq chunks
    nT = S // 128                   # sk tiles
    nD = D // 128                   # contraction tiles

    nc = bacc.Bacc("TRN2", target_bir_lowering=False, debug=False)
    xT = nc.dram_tensor("xT", [D, S], F32R, kind="ExternalInput").ap()
    wqkvT = nc.dram_tensor("wqkvT", [D, M + 2 * HD], F32R, kind="ExternalInput").ap()
    woT = nc.dram_tensor("woT", [M, D], F32R, kind="ExternalInput").ap()
    constD = nc.dram_tensor("constD", [128, 256], F32R, kind="ExternalInput").ap()
    cosP = nc.dram_tensor("cosP", [HD, S], F32, kind="ExternalInput").ap()
    sinP = nc.dram_tensor("sinP", [HD, S], F32, kind="ExternalInput").ap()
    maskD = nc.dram_tensor("maskD", [128, R * ch], F32, kind="ExternalInput").ap()
    outT = nc.dram_tensor("outT", [D, S], F32, kind="ExternalOutput").ap()

    with tile.TileContext(nc) as tc, ExitStack() as ctx, \
            nc.allow_low_precision(reason="fp32r staging for PE matmuls"):
        Exp = mybir.ActivationFunctionType.Exp

        pers = ctx.enter_context(tc.tile_pool(name="pers", bufs=1))
        qT = [pers.tile([HD, S], F32R, tag=f"qT{h}", name=f"qT{h}") for h in range(hq)]
        kT = pers.tile([HD, S], F32R, tag="kT", name="kT")
        vv = pers.tile([128, nT * HD], F32R, tag="vv", name="vv")
        ident = pers.tile([128, 128], F32R, tag="ident", name="ident")
        ones128 = pers.tile([128, 128], F32R, tag="ones128", name="ones128")
        nc.sync.dma_start(ident[:], constD[:, 0:128])
        nc.sync.dma_start(ones128[:], constD[:, 128:256])

        rpool = ctx.enter_context(tc.tile_pool(name="rpool", bufs=2))

        def rope(out, ps, j):
            """out[:,chunk] = RoPE(ps) with de-interleaved halves.

            The 64-partition swap always pairs a PSUM operand with an SBUF
            operand (mixed-space ops may differ in base partition; SB+SB
            ops must not)."""
            cj = cosb[:, j * ch:(j + 1) * ch]
            sj = sinb[:, j * ch:(j + 1) * ch]
            nc.vector.tensor_mul(out, ps[:], cj)
            tmp = rpool.tile([HD, ch], F32, tag="ropetmp", name="ropetmp")
            nc.vector.tensor_mul(tmp[0:H2, :], ps[H2:HD, :], sj[0:H2, :])
            nc.vector.tensor_mul(tmp[H2:HD, :], ps[0:H2, :], sj[H2:HD, :])
            nc.vector.tensor_add(out, out, tmp[:])

        # ---- Phase 1: QKV projections (+RoPE, +v transpose) ----
        # All QKV weights preloaded once (12 MB resident for this phase);
        # re-reading them per sq-chunk made v1 DMA-bound.
        xpool = ctx.enter_context(tc.tile_pool(name="xpool", bufs=8))
        vpool = ctx.enter_context(tc.tile_pool(name="vpool", bufs=2))
        MW = M + 2 * HD
        wqkv_r = wqkvT.rearrange("(d p) m -> p d m", p=128)
        with tc.tile_pool(name="wpool", bufs=1) as wpool, \
             tc.tile_pool(name="qkv_ps", bufs=1, space="PSUM") as qkv_ps, \
             tc.tile_pool(name="vt_ps", bufs=2, space="PSUM") as vt_ps:
            wsb = wpool.tile([128, nD, MW], F32R, tag="wsb", name="wsb")
            H_MW = MW // 2
            for d in range(nD):
                nc.sync.dma_start(wsb[:, d, 0:H_MW], wqkv_r[:, d, 0:H_MW])
                nc.sync.dma_start(wsb[:, d, H_MW:MW], wqkv_r[:, d, H_MW:MW])
            cosb = wpool.tile([HD, S], F32, tag="cosb", name="cosb")
            sinb = wpool.tile([HD, S], F32, tag="sinb", name="sinb")
            nc.sync.dma_start(cosb[:], cosP[:])
            nc.sync.dma_start(sinb[:], sinP[:])
            for j in range(nJ):
                ps_q = [qkv_ps.tile([HD, ch], F32, tag=f"psq{m}", name=f"psq{m}")
                        for m in range(hq)]
                ps_k = qkv_ps.tile([HD, ch], F32, tag="psk", name="psk")
                ps_v = qkv_ps.tile([HD, ch], F32, tag="psv", name="psv")
                for d in range(nD):
                    xt = xpool.tile([128, ch], F32R, tag="xt", name="xt")
                    h2c = ch // 2
                    nc.sync.dma_start(
                        xt[:, 0:h2c],
                        xT[d * 128:(d + 1) * 128, j * ch:j * ch + h2c])
                    nc.sync.dma_start(
                        xt[:, h2c:ch],
                        xT[d * 128:(d + 1) * 128, j * ch + h2c:(j + 1) * ch])
                    st, sp = (d == 0), (d == nD - 1)
                    xr = xt[:]
                    wt = wsb[:, d, :]
                    for m in range(hq):
                        nc.tensor.matmul(
                            ps_q[m][:], wt[:, m * HD:(m + 1) * HD],
                            xr, start=st, stop=sp)
                    nc.tensor.matmul(
                        ps_k[:], wt[:, M:M + HD], xr,
                        start=st, stop=sp)
                    nc.tensor.matmul(
                        ps_v[:], wt[:, M + HD:M + 2 * HD], xr,
                        start=st, stop=sp)
                for m in range(hq):
                    rope(qT[m][:, j * ch:(j + 1) * ch], ps_q[m], j)
                rope(kT[:, j * ch:(j + 1) * ch], ps_k, j)
                # v: psum [hd, ch] -> sbuf, then PE-transpose per 128 block
                vt_s = vpool.tile([HD, ch], F32R, tag="vts", name="vts")
                nc.vector.tensor_copy(vt_s[:], ps_v[:])
                for r in range(R):
                    t = j * R + r
                    pvt = vt_ps.tile([128, 128], F32R, tag="pvt", name="pvt")
                    nc.tensor.transpose(
                        pvt[:], vt_s[:, r * 128:(r + 1) * 128], ident[:])
                    nc.vector.tensor_copy(vv[:, t * HD:(t + 1) * HD], pvt[:])

        # ---- Phases 2+3 share the yT/mask pool (opened after weights free) ----
        ypool = ctx.enter_context(tc.tile_pool(name="ypool", bufs=1))
        yT = [ypool.tile([HD, S], F32R, tag=f"yT{h}", name=f"yT{h}")
              for h in range(hq)]
        maskb = ypool.tile([128, R * ch], F32, tag="maskb", name="maskb")
        nc.sync.dma_start(maskb[:], maskD[:])

        # ---- Phase 2: attention (transposed flash-style, causal) ----
        apool = ctx.enter_context(tc.tile_pool(name="apool", bufs=6))
        npool = ctx.enter_context(tc.tile_pool(name="npool", bufs=2))
        with tc.tile_pool(name="attn_ps", bufs=2, space="PSUM") as attn_ps:
            for h in range(hq):
                for j in range(nJ):
                    nTj = (j + 1) * R   # causal sk-tile count for this chunk
                    y_ps = attn_ps.tile([HD, ch], F32, tag="yps", name="yps")
                    ps_d = attn_ps.tile([128, ch], F32, tag="dps", name="dps",
                                        bufs=2)
                    qslice = qT[h][:, j * ch:(j + 1) * ch]

                    # Software-pipeline the score matmuls LOOK tiles ahead so
                    # the PE stream never parks behind exp_t (ACT latency).
                    LOOK = 2

                    def emit_score(t):
                        s_ps = attn_ps.tile([128, ch], F32, tag="sps",
                                            name="sps", bufs=LOOK + 1)
                        nc.tensor.matmul(
                            s_ps[:], kT[:, t * 128:(t + 1) * 128],
                            qslice, start=True, stop=True)
                        return s_ps

                    pipe = [emit_score(t) for t in range(min(LOOK, nTj))]
                    for t in range(nTj):
                        s_ps = pipe[t]
                        et = apool.tile([128, ch], F32R, tag="exp", name="et")
                        # scale folded into wq host-side; ACT does pure exp
                        nc.scalar.activation(et[:], s_ps[:], Exp)
                        r = t - j * R
                        if r >= 0:  # diagonal tile: apply causal mask
                            nc.vector.tensor_mul(
                                et[:], et[:], maskb[:, r * ch:(r + 1) * ch])
                        if t + LOOK < nTj:
                            pipe.append(emit_score(t + LOOK))
                        # softmax denominator accumulates on PE; all-ones
                        # lhsT broadcasts the column sum to every partition
                        nc.tensor.matmul(
                            ps_d[:], ones128[:], et[:],
                            start=(t == 0), stop=(t == nTj - 1))
                        nc.tensor.matmul(
                            y_ps[:], vv[:, t * HD:(t + 1) * HD],
                            et[:],
                            start=(t == 0), stop=(t == nTj - 1))
                    rec = npool.tile([128, ch], F32, tag="rec", name="rec")
                    nc.vector.reciprocal(rec[:], ps_d[:])
                    nc.vector.tensor_mul(
                        yT[h][:, j * ch:(j + 1) * ch], y_ps[:], rec[:])

        # ---- Phase 3: output projection (row-parallel wo partial sums) ----
        opool = ctx.enter_context(tc.tile_pool(name="opool", bufs=12))
        wopool = ctx.enter_context(tc.tile_pool(name="wopool", bufs=4))
        with tc.tile_pool(name="wo_ps", bufs=1, space="PSUM") as wo_ps:
            for dt in range(nD):
                ps_o = [wo_ps.tile([128, ch], F32, tag=f"pso{j}", name=f"pso{j}",
                                   bufs=2)
                        for j in range(nJ)]
                for o in range(hq):
                    wot = wopool.tile([128, 128], F32R, tag="wot", name="wot")
                    nc.sync.dma_start(
                        wot[:], woT[o * 128:(o + 1) * 128,
                                    dt * 128:(dt + 1) * 128])
                    for j in range(nJ):
                        nc.tensor.matmul(
                            ps_o[j][:], wot[:],
                            yT[o][:, j * ch:(j + 1) * ch],
                            start=(o == 0), stop=(o == hq - 1))
                for j in range(nJ):
                    ot = opool.tile([128, ch], F32, tag="osb", name="osb")
                    nc.vector.tensor_copy(ot[:], ps_o[j][:])
                    h2c = ch // 2
                    nc.sync.dma_start(
                        outT[dt * 128:(dt + 1) * 128, j * ch:j * ch + h2c],
                        ot[:, 0:h2c])
                    nc.sync.dma_start(
                        outT[dt * 128:(dt + 1) * 128,
                             j * ch + h2c:(j + 1) * ch],
                        ot[:, h2c:ch])
    nc.compile()
    return nc


def _deinterleave_perm(hd):
    """Row permutation putting even indices first, odd second."""
    return np.concatenate([np.arange(0, hd, 2), np.arange(1, hd, 2)])


def host_prep(x, wq, wk, wv, wo, freqs_cos, freqs_sin,
              n_cores=N_CORES, hq=HQ, n_kv=N_KV_HEADS):
    """Build the per-core input maps (numpy, host-side)."""
    HD = HEAD_DIM
    D = x.shape[-1]
    S = x.shape[-2]
    M = hq * HD
    R = CH // 128
    x = np.asarray(x, np.float32).reshape(S, D)
    wq = np.asarray(wq, np.float32)
    wk = np.asarray(wk, np.float32)
    wv = np.asarray(wv, np.float32)
    wo = np.asarray(wo, np.float32)
    fc = np.asarray(freqs_cos, np.float32)
    fs = np.asarray(freqs_sin, np.float32)

    perm = _deinterleave_perm(HD)
    wq = wq * np.float32(SCALE)   # fold softmax scale into q projection
    xT = np.ascontiguousarray(x.T)                      # [D, S]
    cosP = np.ascontiguousarray(np.concatenate([fc.T, fc.T], 0))  # [128, S]
    sinP = np.ascontiguousarray(np.concatenate([-fs.T, fs.T], 0))
    # mask[t, r*CH + s] = 1 if 128*r + t <= s else 0
    tt = np.arange(128)[:, None]
    ss = np.arange(CH)[None, :]
    maskD = np.concatenate(
        [(128 * r + tt <= ss).astype(np.float32) for r in range(R)], axis=1)
    maskD = np.ascontiguousarray(maskD)                 # [128, R*CH]
    constD = np.concatenate(
        [np.eye(128, dtype=np.float32), np.ones((128, 128), np.float32)],
        axis=1)                                         # [128, 256]

    in_maps = []
    for c in range(n_cores):
        wq_c = wq[c * M:(c + 1) * M, :].reshape(hq, HD, D)[:, perm, :]
        wq_c = wq_c.reshape(M, D)
        wk_c = wk[c * HD:(c + 1) * HD, :][perm, :]
        wv_c = wv[c * HD:(c + 1) * HD, :]
        wqkvT = np.ascontiguousarray(
            np.concatenate([wq_c, wk_c, wv_c], axis=0).T)  # [D, M+256]
        woT = np.ascontiguousarray(wo[:, c * M:(c + 1) * M].T)  # [M, D]
        in_maps.append({
            "xT": xT, "wqkvT": wqkvT, "woT": woT, "constD": constD,
            "cosP": cosP, "sinP": sinP, "maskD": maskD,
        })
    return in_maps


_NC_CACHE = {}


def _get_module():
    if "nc" not in _NC_CACHE:
        _NC_CACHE["nc"] = build_module()
    return _NC_CACHE["nc"]


def run_on_cores(in_maps, trace=False):
    nc = _get_module()
    res = bass_utils.run_bass_kernel_spmd(
        nc, in_maps, core_ids=list(range(len(in_maps))), trace=trace)
    return res


def kernel(x, wq, wk, wv, wo, freqs_cos, freqs_sin):
    in_maps = host_prep(x, wq, wk, wv, wo, freqs_cos, freqs_sin)
    res = run_on_cores(in_maps)
    acc = None
    for r in res.results:
        o = r["outT"]
        acc = o.astype(np.float64) if acc is None else acc + o
    out = acc.T.astype(np.float32).reshape(1, SEQ, DIM)
    return out



# revision 3
# speedup vs baseline: 1.8267x; 1.8267x over previous
"""Trainium2 Bass kernel for GQA attention (dense_transformer).

Full module: x[1,2048,4096] -> causal GQA attention (32 q heads, 8 kv heads,
head_dim 128, RoPE) -> out[1,2048,4096].

Sharding: tensor-parallel by heads across 8 NeuronCores. Core c owns q heads
4c..4c+3 and kv head c; wq/wk/wv column-sharded, wo row-sharded; x replicated.
The trailing all-reduce over wo partial sums is done host-side (outputs are
gathered to host anyway).

v2 design notes (vs the 3-phase v1 at ~766us):
  - All matmul operands are bf16 (PE runs at the same 1 cycle/row rate as
    fp32r, but DMA bytes, SBUF footprint and DVE op cost halve). PSUM
    accumulation stays fp32; rel-err gate is 2e-2 and bf16 lands ~1e-3.
  - Single fused pass: for each 512-wide seq chunk, QKV projection (+RoPE,
    +V transpose) is immediately followed by causal attention for that chunk
    (k/v of chunks 0..j are ready), with the wo projection at the end. The
    PE never idles long enough to re-throttle (HAM) and phase boundaries
    cost nothing.
  - DMA: ~60 large descriptors (vs 709 small) — weights resident in SBUF,
    x streamed in 1MB quarter-chunks double-buffered by emission order,
    outputs staged in 4-tile groups. Input DMAs ride the sync HWDGE queue;
    small startup tensors + wo weights + output stores ride the scalar
    HWDGE queue so the two FIFOs don't serialize against each other.
  - QKV runs output-major (all 32 d-tiles of one output block back to back)
    so 3 PSUM banks suffice and RoPE consumption trails two banks behind.
  - Attention: scores matmul writes only the causally-valid column range of
    the diagonal tiles; exp is a single ACT op per tile; the softmax
    denominator accumulates via an all-ones PE matmul (broadcast to all
    partitions); 1/d uses the custom-DVE reciprocal_approx_fast (~5x faster
    than the iterative divide, ~18 correct bits).
  - Max-subtraction is skipped: scores are O(+-10), exp cannot overflow.
"""

import math
from contextlib import ExitStack

import numpy as np

import concourse.bass as bass
import concourse.mybir as mybir
import concourse.tile as tile
from concourse import bacc, bass_utils

F32 = mybir.dt.float32
BF16 = mybir.dt.bfloat16

# Full-scale config (hardcoded; kernel.py must be self-contained).
DIM = 4096
SEQ = 2048
N_HEADS = 32
N_KV_HEADS = 8
HEAD_DIM = 128
N_CORES = 8
HQ = N_HEADS // N_CORES            # q heads per core = 4
CH = 512                           # seq chunk
SCALE = 1.0 / math.sqrt(HEAD_DIM)


def build_module(S=SEQ, D=DIM, hq=HQ, ch=CH):
    """Build the SPMD Bass/Tile module for one core's shard."""
    HD = HEAD_DIM
    H2 = HD // 2
    M = hq * HD                     # local q output dim (512)
    MW = M + 2 * HD                 # qkv packed output width (768)
    R = ch // 128                   # sk-tiles per sq chunk (4)
    nJ = S // ch                    # sq chunks (4)
    nT = S // 128                   # sk tiles (16)
    nD = D // 128                   # contraction tiles (32)
    NQ = nD // 4                    # d-tiles per x quarter (8)

    nc = bacc.Bacc("TRN2", target_bir_lowering=False, debug=False)
    xT = nc.dram_tensor("xT", [D, S], BF16, kind="ExternalInput").ap()
    wqkvT = nc.dram_tensor("wqkvT", [D, MW], BF16, kind="ExternalInput").ap()
    woT = nc.dram_tensor("woT", [M, D], BF16, kind="ExternalInput").ap()
    constD = nc.dram_tensor("constD", [128, 256], BF16, kind="ExternalInput").ap()
    cosP = nc.dram_tensor("cosP", [HD, S], BF16, kind="ExternalInput").ap()
    sinP = nc.dram_tensor("sinP", [HD, S], BF16, kind="ExternalInput").ap()
    maskD = nc.dram_tensor("maskD", [128, R * ch], BF16, kind="ExternalInput").ap()
    outT = nc.dram_tensor("outT", [D, S], BF16, kind="ExternalOutput").ap()

    xT_r = xT.rearrange("(g p) s -> p g s", p=128)        # [128, 32, S]
    wqkv_r = wqkvT.rearrange("(d p) m -> p d m", p=128)   # [128, 32, MW]
    woT_r = woT.rearrange("(o p) d -> p o d", p=128)      # [128, 4, D]
    outT_r = outT.rearrange("(g p) s -> p g s", p=128)    # [128, 32, S]

    with tile.TileContext(nc) as tc, ExitStack() as ctx, \
            nc.allow_low_precision(reason="bf16 operands, fp32 accumulation"):
        Exp = mybir.ActivationFunctionType.Exp

        # ---- persistent SBUF ----
        pers = ctx.enter_context(tc.tile_pool(name="pers", bufs=1))
        wsb = pers.tile([128, nD, MW], BF16, tag="wsb", name="wsb")
        wosb = pers.tile([128, hq, D], BF16, tag="wosb", name="wosb")
        kT = pers.tile([HD, S], BF16, tag="kT", name="kT")
        vv = pers.tile([128, nT * HD], BF16, tag="vv", name="vv")
        yT = [pers.tile([HD, S], BF16, tag=f"yT{h}", name=f"yT{h}")
              for h in range(hq)]
        cosb = pers.tile([HD, S], BF16, tag="cosb", name="cosb")
        sinb = pers.tile([HD, S], BF16, tag="sinb", name="sinb")
        maskb = pers.tile([128, R * ch], BF16, tag="maskb", name="maskb")
        ident = pers.tile([128, 128], BF16, tag="ident", name="ident")
        ones128 = pers.tile([128, 128], BF16, tag="ones128", name="ones128")

        # Startup loads on the scalar HWDGE queue (the sync queue carries the
        # weight blocks + x stream; keeping these separate avoids serializing
        # the first chunk behind them).
        nc.scalar.dma_start(cosb[:], cosP[:])
        nc.scalar.dma_start(sinb[:], sinP[:])
        nc.scalar.dma_start(maskb[:], maskD[:])
        nc.scalar.dma_start(ident[:], constD[:, 0:128])
        nc.scalar.dma_start(ones128[:], constD[:, 128:256])

        xpool = ctx.enter_context(tc.tile_pool(name="xpool", bufs=1))
        cur_xq = {}
        for qt in range(4):
            t = xpool.tile([128, NQ, ch], BF16, tag=f"xq{qt}", name=f"xq{qt}")
            nc.scalar.dma_start(t[:], xT_r[:, qt * NQ:(qt + 1) * NQ, 0:ch])
            cur_xq[qt] = t
        nc.scalar.dma_start(wosb[:], woT_r[:])

        # qkv weights on the sync queue, one output block per DMA so the
        # first block's matmuls start ~5us in.
        for o in range(6):
            nc.sync.dma_start(wsb[:, :, o * 128:(o + 1) * 128],
                              wqkv_r[:, :, o * 128:(o + 1) * 128])

        qtpool = ctx.enter_context(tc.tile_pool(name="qtpool", bufs=2))
        rpool = ctx.enter_context(tc.tile_pool(name="rpool", bufs=2))
        vpool = ctx.enter_context(tc.tile_pool(name="vpool", bufs=2))
        apool = ctx.enter_context(tc.tile_pool(name="apool", bufs=6))
        npool = ctx.enter_context(tc.tile_pool(name="npool", bufs=2))

        # Attention PSUM pool is opened first (5 banks: scores 2, y 2, d 1);
        # the qkv pool (3 banks) nests inside and frees its banks to wo_ps.
        attn_ps = ctx.enter_context(
            tc.tile_pool(name="attn_ps", bufs=1, space="PSUM"))

        def rope(out, ps, j):
            """out[:, chunk] = RoPE(ps) with de-interleaved halves.

            The 64-partition swap pairs a PSUM operand with an SBUF operand
            (mixed-space ops may differ in base partition; SB+SB must not).
            """
            cj = cosb[:, j * ch:(j + 1) * ch]
            sj = sinb[:, j * ch:(j + 1) * ch]
            nc.vector.tensor_mul(out, ps[:], cj)
            tmp = rpool.tile([HD, ch], BF16, tag="ropetmp", name="ropetmp")
            nc.vector.tensor_mul(tmp[0:H2, :], ps[H2:HD, :], sj[0:H2, :])
            nc.vector.tensor_mul(tmp[H2:HD, :], ps[0:H2, :], sj[H2:HD, :])
            nc.vector.tensor_add(out, out, tmp[:])

        def qkv_chunk(j, qkv_ps, qTc):
            # Output-major: all 32 d-tile matmuls of one output block run
            # back to back; psum banks rotate a,b,c so RoPE trails 2 behind.
            tags = ["a", "b", "c", "a", "b", "c"]
            for oi in range(6):
                ps = qkv_ps.tile([128, ch], F32, tag=tags[oi], name=f"ps{oi}")
                col = oi * 128
                for dd in range(nD):
                    xt = cur_xq[dd // NQ]
                    nc.tensor.matmul(
                        ps[:], wsb[:, dd, col:col + 128], xt[:, dd % NQ, :],
                        start=(dd == 0), stop=(dd == nD - 1))
                    # v is the last reader of each x quarter: prefetch the
                    # next chunk's quarter right after its final use.
                    if oi == 5 and dd % NQ == NQ - 1 and j + 1 < nJ:
                        qt = dd // NQ
                        nxt = xpool.tile([128, NQ, ch], BF16, tag=f"xq{qt}",
                                         name=f"xq{qt}")
                        nc.sync.dma_start(
                            nxt[:],
                            xT_r[:, qt * NQ:(qt + 1) * NQ,
                                 (j + 1) * ch:(j + 2) * ch])
                        cur_xq[qt] = nxt
                if oi < hq:
                    rope(qTc[oi][:], ps, j)
                elif oi == hq:
                    rope(kT[:, j * ch:(j + 1) * ch], ps, j)
                else:
                    # v: psum [hd, ch] -> sbuf bf16, then PE-transpose per
                    # 128 block into one psum bank (reuses tag "a"), then one
                    # copy into the persistent [sk, hd] v layout.
                    vt_s = vpool.tile([HD, ch], BF16, tag="vts", name="vts")
                    nc.vector.tensor_copy(vt_s[:], ps[:])
                    pvt = qkv_ps.tile([128, ch], BF16, tag="a", name="pvt")
                    for r in range(R):
                        nc.tensor.transpose(
                            pvt[:, r * 128:(r + 1) * 128],
                            vt_s[:, r * 128:(r + 1) * 128], ident[:])
                    nc.vector.tensor_copy(
                        vv[:, j * R * HD:(j + 1) * R * HD], pvt[:])

        def attn_chunk(j, qTc):
            # Transposed flash-style causal attention for the 4 local heads.
            # Scores land transposed (sk on partitions) so P@V needs no
            # transpose; softmax denominator accumulates on the PE via an
            # all-ones lhsT (broadcasts the column sum to every partition).
            nTj = (j + 1) * R
            for h in range(hq):
                y_ps = attn_ps.tile([HD, ch], F32, tag="yps", name="yps",
                                    bufs=2)
                ps_d = attn_ps.tile([128, ch], F32, tag="dps", name="dps")
                qsl = qTc[h]

                def score(t):
                    # Diagonal tiles only have valid scores at sq >= 128*r.
                    off = max(0, (t - j * R) * 128)
                    s_ps = attn_ps.tile([128, ch], F32, tag="sps",
                                        name="sps", bufs=2)
                    nc.tensor.matmul(
                        s_ps[:, off:ch], kT[:, t * 128:(t + 1) * 128],
                        qsl[:, off:ch], start=True, stop=True)
                    return s_ps, off

                pipe = [score(0)]
                for t in range(nTj):
                    s_ps, off = pipe[t]
                    if t + 1 < nTj:
                        pipe.append(score(t + 1))
                    et = apool.tile([128, ch], BF16, tag="et", name="et")
                    # scale folded into wq host-side; ACT does pure exp
                    nc.scalar.activation(et[:, off:ch], s_ps[:, off:ch], Exp)
                    r = t - j * R
                    if r >= 0:  # diagonal tile: apply causal mask
                        nc.vector.tensor_mul(
                            et[:, off:ch], et[:, off:ch],
                            maskb[:, r * ch + off:(r + 1) * ch])
                    st, sp = (t == 0), (t == nTj - 1)
                    nc.tensor.matmul(ps_d[:, off:ch], ones128[:],
                                     et[:, off:ch], start=st, stop=sp)
                    nc.tensor.matmul(y_ps[:, off:ch],
                                     vv[:, t * HD:(t + 1) * HD],
                                     et[:, off:ch], start=st, stop=sp)
                # Copy d out of PSUM promptly (frees the bank for the next
                # head) and take the fast approximate reciprocal in SBUF.
                d_sb = npool.tile([128, ch], F32, tag="dsb", name="dsb")
                nc.vector.tensor_copy(d_sb[:], ps_d[:])
                rec = npool.tile([128, ch], F32, tag="rec", name="rec")
                nc.vector.reciprocal_approx_fast(rec[:], d_sb[:])
                nc.vector.tensor_mul(
                    yT[h][:, j * ch:(j + 1) * ch], y_ps[:], rec[:])

        with tc.tile_pool(name="qkv_ps", bufs=1, space="PSUM") as qkv_ps:
            for j in range(nJ):
                qTc = [qtpool.tile([HD, ch], BF16, tag=f"qt{h}",
                                   name=f"qt{h}") for h in range(hq)]
                qkv_chunk(j, qkv_ps, qTc)
                attn_chunk(j, qTc)

        # ---- output projection (row-parallel wo partial sums) ----
        opool = ctx.enter_context(tc.tile_pool(name="opool", bufs=2))
        with tc.tile_pool(name="wo_ps", bufs=1, space="PSUM") as wo_ps:
            for j in range(nJ):
                for g in range(nD // 4):
                    og = opool.tile([128, 4, ch], BF16, tag="og", name="og")
                    for i in range(4):
                        dt = g * 4 + i
                        ps_o = wo_ps.tile([128, ch], F32, tag="pso",
                                          name="pso", bufs=3)
                        for o in range(hq):
                            nc.tensor.matmul(
                                ps_o[:], wosb[:, o, dt * 128:(dt + 1) * 128],
                                yT[o][:, j * ch:(j + 1) * ch],
                                start=(o == 0), stop=(o == hq - 1))
                        # alternate evacuation between DVE and ACT
                        if dt % 2:
                            nc.scalar.copy(og[:, i, :], ps_o[:])
                        else:
                            nc.vector.tensor_copy(og[:, i, :], ps_o[:])
                    nc.scalar.dma_start(
                        outT_r[:, g * 4:(g + 1) * 4, j * ch:(j + 1) * ch],
                        og[:])
    nc.compile()
    return nc


def _deinterleave_perm(hd):
    """Row permutation putting even indices first, odd second."""
    return np.concatenate([np.arange(0, hd, 2), np.arange(1, hd, 2)])


def host_prep(x, wq, wk, wv, wo, freqs_cos, freqs_sin,
              n_cores=N_CORES, hq=HQ, n_kv=N_KV_HEADS):
    """Build the per-core input maps (numpy, host-side)."""
    import ml_dtypes

    BF = ml_dtypes.bfloat16
    HD = HEAD_DIM
    D = x.shape[-1]
    S = x.shape[-2]
    M = hq * HD
    R = CH // 128
    x = np.asarray(x, np.float32).reshape(S, D)
    wq = np.asarray(wq, np.float32)
    wk = np.asarray(wk, np.float32)
    wv = np.asarray(wv, np.float32)
    wo = np.asarray(wo, np.float32)
    fc = np.asarray(freqs_cos, np.float32)
    fs = np.asarray(freqs_sin, np.float32)

    perm = _deinterleave_perm(HD)
    wq = wq * np.float32(SCALE)   # fold softmax scale into q projection
    xT = np.ascontiguousarray(x.T).astype(BF)                 # [D, S]
    cosP = np.ascontiguousarray(np.concatenate([fc.T, fc.T], 0)).astype(BF)
    sinP = np.ascontiguousarray(np.concatenate([-fs.T, fs.T], 0)).astype(BF)
    # mask[t, r*CH + s] = 1 if 128*r + t <= s else 0
    tt = np.arange(128)[:, None]
    ss = np.arange(CH)[None, :]
    maskD = np.concatenate(
        [(128 * r + tt <= ss).astype(np.float32) for r in range(R)], axis=1)
    maskD = np.ascontiguousarray(maskD).astype(BF)            # [128, R*CH]
    constD = np.concatenate(
        [np.eye(128, dtype=np.float32), np.ones((128, 128), np.float32)],
        axis=1).astype(BF)                                    # [128, 256]

    in_maps = []
    for c in range(n_cores):
        wq_c = wq[c * M:(c + 1) * M, :].reshape(hq, HD, D)[:, perm, :]
        wq_c = wq_c.reshape(M, D)
        wk_c = wk[c * HD:(c + 1) * HD, :][perm, :]
        wv_c = wv[c * HD:(c + 1) * HD, :]
        wqkvT = np.ascontiguousarray(
            np.concatenate([wq_c, wk_c, wv_c], axis=0).T).astype(BF)
        woT = np.ascontiguousarray(wo[:, c * M:(c + 1) * M].T).astype(BF)
        in_maps.append({
            "xT": xT, "wqkvT": wqkvT, "woT": woT, "constD": constD,
            "cosP": cosP, "sinP": sinP, "maskD": maskD,
        })
    return in_maps


_NC_CACHE = {}


def _get_module():
    if "nc" not in _NC_CACHE:
        _NC_CACHE["nc"] = build_module()
    return _NC_CACHE["nc"]


def run_on_cores(in_maps, trace=False):
    nc = _get_module()
    res = bass_utils.run_bass_kernel_spmd(
        nc, in_maps, core_ids=list(range(len(in_maps))), trace=trace)
    return res


def kernel(x, wq, wk, wv, wo, freqs_cos, freqs_sin):
    in_maps = host_prep(x, wq, wk, wv, wo, freqs_cos, freqs_sin)
    res = run_on_cores(in_maps)
    acc = None
    for r in res.results:
        o = np.asarray(r["outT"], dtype=np.float64)
        acc = o if acc is None else acc + o
    out = acc.T.astype(np.float32).reshape(1, SEQ, DIM)
    return out
